# revision 22
# baseline (speedup 1.0000x reference)
"""Trainium2 Bass kernel for the HNN leapfrog integrator (nn_HNN_39968965657036).

Data-parallel over batch: 8192 samples -> 8 cores x 1024. All weights and
state SBUF-resident; 16 leapfrog steps x 2 gradient evals run fully on-chip.

v5b: all four matmul layers in fp8 DoubleRow; psum allocated as [128,1024]
double tiles pairing the two m-chunks of each DR weight tile, so every
elementwise psum drain is a single 1024-wide op (amortizes the fixed per-op
overhead and halves sync traffic). Activation tensors use a batch-major DR
layout [ki, b*1024 + o*512 + n] so drain destinations are contiguous.
Scalar engine: relu + sigmoid-step masks; Vector: g1 mask-mult and the
state update chain (fp8 state one hop after L4, f32 master deferred).
"""
import numpy as np
from contextlib import ExitStack

import concourse.bass as bass
import concourse.mybir as mybir
import concourse.tile as tile
from concourse.masks import make_identity

D = 256          # hnn dim; state dim = 2D = 512
F = 2 * D        # 512 features
STEPS = 16
DT = 0.1
NCORES = 8
BCORE = 1024     # batch per core
NBH = 2          # batch halves per core
BH = BCORE // NBH  # 512 = moving-operand width
P = 128
FC = F // P      # 4 feature chunks
BC = BCORE // P  # 8 batch chunks

f32 = mybir.dt.float32
fp8 = mybir.dt.float8e4

SW = 16.0            # fp8 scale on W1 / W2 (keeps entries out of subnormals)
S3 = 512.0           # fp8 scale on Wo-folded W2 (L3 stationary)
K_P = -0.5 * DT / (SW * S3)   # L4 psum -> p half-kick coefficient
K_Q = DT / (SW * S3)          # L4 psum -> q drift coefficient
SIG = 2.0 ** 20      # sigmoid(SIG*x) == exact (x>0) step after fp8 rounding


def _split_multi_waits(nc):
    """walrus codegen allows at most ONE sync wait per instruction; hoist
    extras onto preceding single-wait NoOps on the same engine queue."""
    skip = {"InstAllEngineBarrier", "InstEventSemaphore"}
    ctr = 0
    for f in nc.m.functions:
        for blk in f.blocks:
            out = []
            changed = False
            for inst in blk.instructions:
                si = inst.sync_info
                if (si is not None and si.on_wait and len(si.on_wait) > 1
                        and type(inst).__name__ not in skip):
                    waits = list(si.on_wait)
                    for w in waits[:-1]:
                        ctr += 1
                        nop = mybir.InstNoOp(name=f"I-wsplit-{ctr}", ins=[], outs=[])
                        nop.engine = inst.engine
                        nop.sync_info = mybir.SyncInfo(on_wait=[w], on_update=[])
                        out.append(nop)
                    inst.sync_info = mybir.SyncInfo(
                        on_wait=[waits[-1]], on_update=list(si.on_update or []))
                    changed = True
                out.append(inst)
            if changed:
                blk.instructions = out
    return ctr


def _build():
    nc = bass.Bass(trn_type="TRN2")
    X = nc.dram_tensor("x", [BCORE, F * 2], f32, kind="ExternalInput")   # [1024, 1024]
    W1d = nc.dram_tensor("w1", [F, F], f32, kind="ExternalInput")
    W2d = nc.dram_tensor("w2", [F, F], f32, kind="ExternalInput")
    Wod = nc.dram_tensor("wo", [1, F], f32, kind="ExternalInput")
    OUT = nc.dram_tensor("out", [BCORE, F], f32, kind="ExternalOutput")

    AF = mybir.ActivationFunctionType
    ALU = mybir.AluOpType
    DR = mybir.MatmulPerfMode.DoubleRow

    with tile.TileContext(nc) as tc, ExitStack() as ctx:
        sb = ctx.enter_context(tc.tile_pool(name="sb", bufs=1))
        ps = ctx.enter_context(tc.tile_pool(name="ps", bufs=4, space="PSUM"))

        def psum2():
            # [128, 1024] f32 double tile = 2 psum banks; 4 rotating = 8 banks
            return ps.tile([P, 2 * BH], f32, tag="mm", bufs=4, name="pmm")

        # ---------------- load ----------------
        # w1 first (gates the first PE transposes), then x batch-half 0
        # (gates step0's L1(b0)), then w2, then the rest of x
        w1_sb = [sb.tile([P, F], f32, tag=f"w1_{k}", name=f"w1_{k}") for k in range(FC)]
        w2_sb = [sb.tile([P, F], f32, tag=f"w2_{k}", name=f"w2_{k}") for k in range(FC)]
        x_sb = [sb.tile([P, F * 2], f32, tag=f"x{c}", name=f"x{c}") for c in range(BC)]
        for k in range(FC):
            nc.sync.dma_start(w1_sb[k][:], W1d[k * P:(k + 1) * P, :])
        for c in range(BC // 2):
            nc.sync.dma_start(x_sb[c][:], X[c * P:(c + 1) * P, :])
        for k in range(FC):
            nc.sync.dma_start(w2_sb[k][:], W2d[k * P:(k + 1) * P, :])
        for c in range(BC // 2, BC):
            nc.sync.dma_start(x_sb[c][:], X[c * P:(c + 1) * P, :])
        woT = [sb.tile([P, 1], f32, tag=f"wo{k}", name=f"wo{k}") for k in range(FC)]
        for k in range(FC):
            nc.sync.dma_start(woT[k][:], Wod[:, k * P:(k + 1) * P])

        ident = sb.tile([P, P], f32, tag="ident")
        make_identity(nc, ident[:])

        # ---------------- weight prep: fp8 DoubleRow stationaries ----------
        # DR layout pairs feature chunks (2j, 2j+1): tile[ki, o*W + m] holds
        # element [feature f = j*256 + o*128 + ki, m].
        w1T_dr = [sb.tile([P, 2 * F], fp8, tag=f"w1T{j}", name=f"w1T{j}") for j in range(2)]
        w2T_dr = [sb.tile([P, 2 * F], fp8, tag=f"w2T{j}", name=f"w2T{j}") for j in range(2)]
        for fc in range(FC):           # source column chunk of W (feature f)
            j, o = fc // 2, fc % 2
            for mc in range(FC):       # source row chunk of W (output m)
                pt = psum2()
                nc.tensor.transpose(pt[:, :P], w1_sb[mc][:, fc * P:(fc + 1) * P],
                                    ident[:])
                nc.scalar.activation(
                    w1T_dr[j][:, o * F + mc * P:o * F + (mc + 1) * P],
                    pt[:, :P], AF.Copy, scale=SW)
                pt2 = psum2()
                nc.tensor.transpose(pt2[:, :P], w2_sb[mc][:, fc * P:(fc + 1) * P],
                                    ident[:])
                nc.scalar.activation(
                    w2T_dr[j][:, o * F + mc * P:o * F + (mc + 1) * P],
                    pt2[:, :P], AF.Copy, scale=SW)
        # L3 lhsT: w2w_dr[j][ki, o*F+i] = Wo[f]*W2[f, i]*S3 (f = row index)
        w2w_dr = [sb.tile([P, 2 * F], fp8, tag=f"w2w{j}", name=f"w2w{j}")
                  for j in range(2)]
        for c in range(FC):
            j, o = c // 2, c % 2
            nc.vector.tensor_scalar(w2w_dr[j][:, o * F:(o + 1) * F], w2_sb[c][:],
                                    woT[c][:], S3, ALU.mult, ALU.mult)
        # L4 lhsT: w1_dr[j][ki, o*F+m] = W1[f, m] * SW (f = row index)
        w1_dr = [sb.tile([P, 2 * F], fp8, tag=f"w1f{j}", name=f"w1f{j}")
                 for j in range(2)]
        for c in range(FC):
            j, o = c // 2, c % 2
            nc.vector.tensor_scalar_mul(w1_dr[j][:, o * F:(o + 1) * F],
                                        w1_sb[c][:], SW)

        # ------- input prep: q = x[:,:,3], p = x[:,:,3]-x[:,:,2] ------------
        # masters, batch-major: [ki, b*1024 + mloc*512 + n] = state[mloc*128+ki,
        # b*512 + n]; matches st_dr's fp8 DR layout element-for-element.
        qM = sb.tile([P, 2 * BCORE], f32, tag="qM", name="qM")
        pM = sb.tile([P, 2 * BCORE], f32, tag="pM", name="pM")
        for c in range(BC):
            b, cp = c // (BC // 2), c % (BC // 2)
            xv = x_sb[c][:].rearrange("p (f c) -> p f c", c=4)
            qb = sb.tile([P, D], f32, tag="qb", bufs=3)
            pb = sb.tile([P, D], f32, tag="pb", bufs=3)
            nc.vector.tensor_copy(qb[:], xv[:, :, 3])
            nc.vector.tensor_tensor(pb[:], xv[:, :, 3], xv[:, :, 2],
                                    ALU.subtract)
            for m in range(D // P):
                col = b * BCORE + m * BH + cp * P
                pt = psum2()
                nc.tensor.transpose(pt[:, :P], qb[:, m * P:(m + 1) * P], ident[:])
                nc.scalar.copy(qM[:, col:col + P], pt[:, :P])
                pt2 = psum2()
                nc.tensor.transpose(pt2[:, :P], pb[:, m * P:(m + 1) * P], ident[:])
                nc.scalar.copy(pM[:, col:col + P], pt2[:, :P])

        # fp8 state in DR layout: st_dr[0] = q chunks, st_dr[1] = p chunks
        # (filled per batch-half so step0's L1(b0) starts on half the input)
        st_dr = [sb.tile([P, 2 * BCORE], fp8, tag=f"st{j}", name=f"st{j}")
                 for j in range(2)]
        for b in range(NBH):
            bs = slice(b * BCORE, (b + 1) * BCORE)
            nc.scalar.copy(st_dr[0][:, bs], qM[:, bs])
            nc.scalar.copy(st_dr[1][:, bs], pM[:, bs])

        a1_dr = [sb.tile([P, 2 * BCORE], fp8, tag=f"a1_{j}", name=f"a1_{j}")
                 for j in range(2)]
        m2_dr = [sb.tile([P, 2 * BCORE], fp8, tag=f"m2_{j}", name=f"m2_{j}")
                 for j in range(2)]
        g1_dr = [sb.tile([P, 2 * BCORE], fp8, tag=f"g1_{j}", name=f"g1_{j}")
                 for j in range(2)]

        def scr_tile():
            # f32 staging for k*psum so the master add runs off-psum on gpsimd
            return sb.tile([P, 2 * BH], f32, tag="scr", bufs=4, name="scr")

        # ---------------- 16 leapfrog steps ----------------
        def mm_pair(lhsT_dr, rhs_dr, jp, b):
            """One [128,1024] double psum: output chunks m=2jp, 2jp+1 for
            batch half b, each over the full 512 contraction (2 DR MMs)."""
            pt = psum2()
            bs = slice(b * BCORE, (b + 1) * BCORE)
            for o in range(2):
                m = 2 * jp + o
                half = pt[:, o * BH:(o + 1) * BH]
                for jc in range(2):
                    lhsT = lhsT_dr[jc][:].rearrange("p (o m) -> p o m", o=2)[
                        :, :, m * P:(m + 1) * P]
                    rhs = rhs_dr[jc][:, bs].rearrange("p (o n) -> p o n", o=2)
                    nc.tensor.matmul(half, lhsT, rhs, start=(jc == 0),
                                     stop=(jc == 1), perf_mode=DR)
            return pt

        def grad_eval(full):
            """One gradient eval; full=True also produces q updates."""
            for b in range(NBH):
                bs = slice(b * BCORE, (b + 1) * BCORE)
                for jp in range(2):  # L1: h1.T = W1 @ state.T (psum = SW*h1)
                    pt = mm_pair(w1T_dr, st_dr, jp, b)
                    if b == 0 and jp == 1:  # one per eval on DVE: unclogs the
                        # ACT relu chain that gates L2's matmuls
                        nc.vector.tensor_scalar(a1_dr[jp][:, bs], pt[:],
                                                1.0 / SW, 0.0, ALU.mult,
                                                ALU.max)
                    else:
                        nc.scalar.activation(a1_dr[jp][:, bs], pt[:], AF.Relu,
                                             scale=1.0 / SW)
            for b in range(NBH):
                bs = slice(b * BCORE, (b + 1) * BCORE)
                for jp in range(2):  # L2: h2.T = W2 @ a1.T (sign only)
                    pt = mm_pair(w2T_dr, a1_dr, jp, b)
                    if b == 1 and jp == 1:  # one per eval on DVE for balance
                        nc.vector.tensor_scalar(m2_dr[jp][:, bs], pt[:], 0.0,
                                                None, ALU.is_gt)
                    else:
                        # sigmoid(2^20 x) == exact 0/1 step after fp8 rounding
                        nc.scalar.activation(m2_dr[jp][:, bs], pt[:], AF.Sigmoid,
                                             scale=SIG)
            for b in range(NBH):
                bs = slice(b * BCORE, (b + 1) * BCORE)
                for jp in range(2):  # L3: u.T = (Wo*W2).T @ m2.T (psum = S3*u)
                    pt = mm_pair(w2w_dr, m2_dr, jp, b)
                    # g1 = (a1 > 0) * u ; b1's drains split in halves so the
                    # first half lands sooner for L4's matmuls
                    if b == 1:
                        for o in range(2):
                            hs = slice(b * BCORE + o * BH,
                                       b * BCORE + (o + 1) * BH)
                            nc.vector.scalar_tensor_tensor(
                                g1_dr[jp][:, hs], a1_dr[jp][:, hs], 0.0,
                                pt[:, o * BH:(o + 1) * BH], ALU.is_gt, ALU.mult)
                    else:
                        nc.vector.scalar_tensor_tensor(g1_dr[jp][:, bs],
                                                       a1_dr[jp][:, bs], 0.0,
                                                       pt[:], ALU.is_gt,
                                                       ALU.mult)
            addbacks = []
            for b in range(NBH):
                bs = slice(b * BCORE, (b + 1) * BCORE)
                for jp in range(2) if full else range(1):
                    # L4: dH.T = W1.T @ g1.T (psum = SW*S3*dH)
                    # jp 0: dH_q -> p half-kick ; jp 1: dH_p -> q drift
                    pt = mm_pair(w1_dr, g1_dr, jp, b)
                    if jp == 0:
                        tgt, k, sj = pM, K_P, 1
                    else:
                        tgt, k, sj = qM, K_Q, 0
                    # critical path: fp8 state for the next eval's L1, one hop
                    nc.vector.scalar_tensor_tensor(st_dr[sj][:, bs], pt[:], k,
                                                   tgt[:, bs], ALU.mult, ALU.add)
                    # k*psum staged to SBUF (frees the psum bank quickly);
                    # ACT is idle in this phase, DVE takes one per half
                    sc = scr_tile()
                    if b == 0 or jp == 1:
                        nc.scalar.activation(sc[:], pt[:], AF.Copy, scale=k)
                    else:
                        nc.vector.tensor_scalar_mul(sc[:], pt[:], k)
                    addbacks.append((sc, tgt, bs))
            # f32 master updates off both hot engines (SBUF-only on gpsimd);
            # p first: its result is read one eval sooner than q's
            for sc, tgt, bs in sorted(addbacks, key=lambda t: t[1] is qM):
                nc.gpsimd.tensor_tensor(tgt[:, bs], tgt[:, bs], sc[:], ALU.add)

        out_sb = [sb.tile([P, F], f32, tag=f"ob{c}", name=f"ob{c}") for c in range(BC)]

        def emit_out(src, col0, split=False):
            idx = 0
            for b in range(NBH):
                for cp in range(BC // 2):
                    c = b * (BC // 2) + cp
                    for m in range(D // P):
                        pt = psum2()
                        scol = b * BCORE + m * BH + cp * P
                        nc.tensor.transpose(pt[:, :P], src[:, scol:scol + P],
                                            ident[:])
                        dst = out_sb[c][:, col0 + m * P:col0 + (m + 1) * P]
                        if split and idx % 2:
                            nc.vector.tensor_copy(dst, pt[:, :P])
                        else:
                            nc.scalar.copy(dst, pt[:, :P])
                        idx += 1

        for step in range(STEPS):
            with nc.named_scope(f"step{step}"):
                # eval A: updates p (half-kick) and q (drift)
                grad_eval(full=True)
                if step == STEPS - 1:
                    # q is final after the drift; transpose it out and start
                    # its DMA while the last eval (p-only) runs
                    emit_out(qM, 0)
                    for c in range(BC):
                        nc.sync.dma_start(OUT[c * P:(c + 1) * P, :D],
                                          out_sb[c][:, :D])
                # eval B: second half-kick on p only
                grad_eval(full=False)

        # ---------------- output: out = concat([q, p], -1), batch-major ------
        emit_out(pM, D, split=True)
        for c in range(BC):
            nc.sync.dma_start(OUT[c * P:(c + 1) * P, D:], out_sb[c][:, D:])

    _split_multi_waits(nc)
    return nc


_CACHE = {}


def _get_nc():
    if "nc" not in _CACHE:
        _CACHE["nc"] = _build()
    return _CACHE["nc"]


def kernel(x, W1, b1, W2, b2, Wo, _trace=False):
    from concourse.bass_utils import run_bass_kernel_spmd
    nc = _get_nc()
    x = np.ascontiguousarray(np.asarray(x, dtype=np.float32))
    W1 = np.ascontiguousarray(np.asarray(W1, dtype=np.float32))
    W2 = np.ascontiguousarray(np.asarray(W2, dtype=np.float32))
    Wo = np.ascontiguousarray(np.asarray(Wo, dtype=np.float32))
    B = x.shape[0]
    xf = x.reshape(NCORES, BCORE, F * 2)
    in_maps = [
        {"x": np.ascontiguousarray(xf[c]), "w1": W1, "w2": W2, "wo": Wo}
        for c in range(NCORES)
    ]
    res = run_bass_kernel_spmd(nc, in_maps, core_ids=list(range(NCORES)),
                               trace=_trace)
    out = np.concatenate([r["out"] for r in res.results], axis=0)
    if _trace:
        kernel.last_result = res
    return out


# revision 28
# speedup vs baseline: 1.0134x; 1.0134x over previous
"""Trainium2 Bass kernel for the HNN leapfrog integrator (nn_HNN_39968965657036).

Data-parallel over batch: 8192 samples -> 8 cores x 1024. All weights and
state SBUF-resident; 16 leapfrog steps x 2 gradient evals run fully on-chip.

All four matmul layers run in fp8 DoubleRow (2x contraction per MM
instruction; the gradient depends on the state only through the relu masks,
so fp8 forward error only flips near-zero mask bits). PSUM is allocated as
[128,1024] double tiles pairing the two m-chunks of each DR weight tile, so
every elementwise psum drain is a single 1024-wide op. Activation tensors
use a batch-major DR layout [ki, b*1024 + o*512 + n] so drain destinations
are contiguous. Engine balance: Scalar does relu + sigmoid-step masks +
k*psum staging copies; Vector does the g1 mask-mult and the one-hop fp8
state updates; GpSimd applies the deferred f32 master accumulations
(SBUF-only, off both hot engines).
"""
import numpy as np
from contextlib import ExitStack

import concourse.bass as bass
import concourse.mybir as mybir
import concourse.tile as tile
from concourse.masks import make_identity

D = 256          # hnn dim; state dim = 2D = 512
F = 2 * D        # 512 features
STEPS = 16
DT = 0.1
NCORES = 8
BCORE = 1024     # batch per core
NBH = 2          # batch halves per core
BH = BCORE // NBH  # 512 = moving-operand width
P = 128
FC = F // P      # 4 feature chunks
BC = BCORE // P  # 8 batch chunks

f32 = mybir.dt.float32
fp8 = mybir.dt.float8e4

SW = 16.0            # fp8 scale on W1 / W2 (keeps entries out of subnormals)
S3 = 512.0           # fp8 scale on Wo-folded W2 (L3 stationary)
K_P = -0.5 * DT / (SW * S3)   # L4 psum -> p half-kick coefficient
K_Q = DT / (SW * S3)          # L4 psum -> q drift coefficient
SIG = 2.0 ** 20      # sigmoid(SIG*x) == exact (x>0) step after fp8 rounding


def _split_multi_waits(nc):
    """walrus codegen allows at most ONE sync wait per instruction; hoist
    extras onto preceding single-wait NoOps on the same engine queue."""
    skip = {"InstAllEngineBarrier", "InstEventSemaphore"}
    ctr = 0
    for f in nc.m.functions:
        for blk in f.blocks:
            out = []
            changed = False
            for inst in blk.instructions:
                si = inst.sync_info
                if (si is not None and si.on_wait and len(si.on_wait) > 1
                        and type(inst).__name__ not in skip):
                    waits = list(si.on_wait)
                    for w in waits[:-1]:
                        ctr += 1
                        nop = mybir.InstNoOp(name=f"I-wsplit-{ctr}", ins=[], outs=[])
                        nop.engine = inst.engine
                        nop.sync_info = mybir.SyncInfo(on_wait=[w], on_update=[])
                        out.append(nop)
                    inst.sync_info = mybir.SyncInfo(
                        on_wait=[waits[-1]], on_update=list(si.on_update or []))
                    changed = True
                out.append(inst)
            if changed:
                blk.instructions = out
    return ctr


def _build():
    nc = bass.Bass(trn_type="TRN2")
    X = nc.dram_tensor("x", [BCORE, F * 2], f32, kind="ExternalInput")   # [1024, 1024]
    W1d = nc.dram_tensor("w1", [F, F], f32, kind="ExternalInput")
    W2d = nc.dram_tensor("w2", [F, F], f32, kind="ExternalInput")
    Wod = nc.dram_tensor("wo", [1, F], f32, kind="ExternalInput")
    OUT = nc.dram_tensor("out", [BCORE, F], f32, kind="ExternalOutput")

    AF = mybir.ActivationFunctionType
    ALU = mybir.AluOpType
    DR = mybir.MatmulPerfMode.DoubleRow

    with tile.TileContext(nc) as tc, ExitStack() as ctx:
        sb = ctx.enter_context(tc.tile_pool(name="sb", bufs=1))
        ps = ctx.enter_context(tc.tile_pool(name="ps", bufs=4, space="PSUM"))

        def psum2():
            # [128, 1024] f32 double tile = 2 psum banks; 4 rotating = 8 banks
            return ps.tile([P, 2 * BH], f32, tag="mm", bufs=4, name="pmm")

        # ---------------- load ----------------
        # weights first: PE's first work (weight transposes) depends on them
        w1_sb = [sb.tile([P, F], f32, tag=f"w1_{k}", name=f"w1_{k}") for k in range(FC)]
        w2_sb = [sb.tile([P, F], f32, tag=f"w2_{k}", name=f"w2_{k}") for k in range(FC)]
        for k in range(FC):
            nc.sync.dma_start(w1_sb[k][:], W1d[k * P:(k + 1) * P, :])
            nc.sync.dma_start(w2_sb[k][:], W2d[k * P:(k + 1) * P, :])
        woT = [sb.tile([P, 1], f32, tag=f"wo{k}", name=f"wo{k}") for k in range(FC)]
        for k in range(FC):
            nc.sync.dma_start(woT[k][:], Wod[:, k * P:(k + 1) * P])
        x_sb = [sb.tile([P, F * 2], f32, tag=f"x{c}", name=f"x{c}") for c in range(BC)]
        for c in range(BC):
            nc.sync.dma_start(x_sb[c][:], X[c * P:(c + 1) * P, :])

        ident = sb.tile([P, P], f32, tag="ident")
        make_identity(nc, ident[:])

        # ---------------- weight prep: fp8 DoubleRow stationaries ----------
        # DR layout pairs feature chunks (2j, 2j+1): tile[ki, o*W + m] holds
        # element [feature f = j*256 + o*128 + ki, m].
        w1T_dr = [sb.tile([P, 2 * F], fp8, tag=f"w1T{j}", name=f"w1T{j}") for j in range(2)]
        w2T_dr = [sb.tile([P, 2 * F], fp8, tag=f"w2T{j}", name=f"w2T{j}") for j in range(2)]
        for fc in range(FC):           # source column chunk of W (feature f)
            j, o = fc // 2, fc % 2
            for mc in range(FC):       # source row chunk of W (output m)
                pt = psum2()
                nc.tensor.transpose(pt[:, :P], w1_sb[mc][:, fc * P:(fc + 1) * P],
                                    ident[:])
                nc.scalar.activation(
                    w1T_dr[j][:, o * F + mc * P:o * F + (mc + 1) * P],
                    pt[:, :P], AF.Copy, scale=SW)
                pt2 = psum2()
                nc.tensor.transpose(pt2[:, :P], w2_sb[mc][:, fc * P:(fc + 1) * P],
                                    ident[:])
                nc.scalar.activation(
                    w2T_dr[j][:, o * F + mc * P:o * F + (mc + 1) * P],
                    pt2[:, :P], AF.Copy, scale=SW)
        # L3 lhsT: w2w_dr[j][ki, o*F+i] = Wo[f]*W2[f, i]*S3 (f = row index)
        w2w_dr = [sb.tile([P, 2 * F], fp8, tag=f"w2w{j}", name=f"w2w{j}")
                  for j in range(2)]
        for c in range(FC):
            j, o = c // 2, c % 2
            nc.vector.tensor_scalar(w2w_dr[j][:, o * F:(o + 1) * F], w2_sb[c][:],
                                    woT[c][:], S3, ALU.mult, ALU.mult)
        # L4 lhsT: w1_dr[j][ki, o*F+m] = W1[f, m] * SW (f = row index)
        w1_dr = [sb.tile([P, 2 * F], fp8, tag=f"w1f{j}", name=f"w1f{j}")
                 for j in range(2)]
        for c in range(FC):
            j, o = c // 2, c % 2
            nc.vector.tensor_scalar_mul(w1_dr[j][:, o * F:(o + 1) * F],
                                        w1_sb[c][:], SW)

        # ------- input prep: q = x[:,:,3], p = x[:,:,3]-x[:,:,2] ------------
        # masters, batch-major: [ki, b*1024 + mloc*512 + n] = state[mloc*128+ki,
        # b*512 + n]; matches st_dr's fp8 DR layout element-for-element.
        qM = sb.tile([P, 2 * BCORE], f32, tag="qM", name="qM")
        pM = sb.tile([P, 2 * BCORE], f32, tag="pM", name="pM")
        for c in range(BC):
            b, cp = c // (BC // 2), c % (BC // 2)
            xv = x_sb[c][:].rearrange("p (f c) -> p f c", c=4)
            qb = sb.tile([P, D], f32, tag="qb", bufs=3)
            pb = sb.tile([P, D], f32, tag="pb", bufs=3)
            nc.vector.tensor_copy(qb[:], xv[:, :, 3])
            nc.vector.tensor_tensor(pb[:], xv[:, :, 3], xv[:, :, 2],
                                    ALU.subtract)
            for m in range(D // P):
                col = b * BCORE + m * BH + cp * P
                pt = psum2()
                nc.tensor.transpose(pt[:, :P], qb[:, m * P:(m + 1) * P], ident[:])
                nc.scalar.copy(qM[:, col:col + P], pt[:, :P])
                pt2 = psum2()
                nc.tensor.transpose(pt2[:, :P], pb[:, m * P:(m + 1) * P], ident[:])
                nc.scalar.copy(pM[:, col:col + P], pt2[:, :P])

        # fp8 state in DR layout: st_dr[0] = q chunks, st_dr[1] = p chunks
        # (filled per batch-half so step0's L1(b0) starts on half the input)
        st_dr = [sb.tile([P, 2 * BCORE], fp8, tag=f"st{j}", name=f"st{j}")
                 for j in range(2)]
        for b in range(NBH):
            bs = slice(b * BCORE, (b + 1) * BCORE)
            nc.scalar.copy(st_dr[0][:, bs], qM[:, bs])
            nc.scalar.copy(st_dr[1][:, bs], pM[:, bs])

        a1_dr = [sb.tile([P, 2 * BCORE], fp8, tag=f"a1_{j}", name=f"a1_{j}")
                 for j in range(2)]
        m2_dr = [sb.tile([P, 2 * BCORE], fp8, tag=f"m2_{j}", name=f"m2_{j}")
                 for j in range(2)]
        g1_dr = [sb.tile([P, 2 * BCORE], fp8, tag=f"g1_{j}", name=f"g1_{j}")
                 for j in range(2)]

        def scr_tile():
            # f32 staging for k*psum so the master add runs off-psum on gpsimd
            return sb.tile([P, 2 * BH], f32, tag="scr", bufs=4, name="scr")

        # ---------------- 16 leapfrog steps ----------------
        def mm_pair(lhsT_dr, rhs_dr, jp, b):
            """One [128,1024] double psum: output chunks m=2jp, 2jp+1 for
            batch half b, each over the full 512 contraction (2 DR MMs)."""
            pt = psum2()
            bs = slice(b * BCORE, (b + 1) * BCORE)
            for o in range(2):
                m = 2 * jp + o
                half = pt[:, o * BH:(o + 1) * BH]
                for jc in range(2):
                    lhsT = lhsT_dr[jc][:].rearrange("p (o m) -> p o m", o=2)[
                        :, :, m * P:(m + 1) * P]
                    rhs = rhs_dr[jc][:, bs].rearrange("p (o n) -> p o n", o=2)
                    nc.tensor.matmul(half, lhsT, rhs, start=(jc == 0),
                                     stop=(jc == 1), perf_mode=DR)
            return pt

        def grad_eval(full):
            """One gradient eval; full=True also produces q updates."""
            for b in range(NBH):
                bs = slice(b * BCORE, (b + 1) * BCORE)
                for jp in range(2):  # L1: h1.T = W1 @ state.T (psum = SW*h1)
                    pt = mm_pair(w1T_dr, st_dr, jp, b)
                    nc.scalar.activation(a1_dr[jp][:, bs], pt[:], AF.Relu,
                                         scale=1.0 / SW)
            for b in range(NBH):
                bs = slice(b * BCORE, (b + 1) * BCORE)
                for jp in range(2):  # L2: h2.T = W2 @ a1.T (sign only)
                    pt = mm_pair(w2T_dr, a1_dr, jp, b)
                    if b == 1 and jp == 1:  # one per eval on DVE for balance
                        nc.vector.tensor_scalar(m2_dr[jp][:, bs], pt[:], 0.0,
                                                None, ALU.is_gt)
                    else:
                        # sigmoid(2^20 x) == exact 0/1 step after fp8 rounding
                        nc.scalar.activation(m2_dr[jp][:, bs], pt[:], AF.Sigmoid,
                                             scale=SIG)
            for b in range(NBH):
                bs = slice(b * BCORE, (b + 1) * BCORE)
                for jp in range(2):  # L3: u.T = (Wo*W2).T @ m2.T (psum = S3*u)
                    pt = mm_pair(w2w_dr, m2_dr, jp, b)
                    # g1 = (a1 > 0) * u
                    nc.vector.scalar_tensor_tensor(g1_dr[jp][:, bs],
                                                   a1_dr[jp][:, bs], 0.0, pt[:],
                                                   ALU.is_gt, ALU.mult)
            addbacks = []
            for b in range(NBH):
                bs = slice(b * BCORE, (b + 1) * BCORE)
                for jp in range(2) if full else range(1):
                    # L4: dH.T = W1.T @ g1.T (psum = SW*S3*dH)
                    # jp 0: dH_q -> p half-kick ; jp 1: dH_p -> q drift
                    pt = mm_pair(w1_dr, g1_dr, jp, b)
                    if jp == 0:
                        tgt, k, sj = pM, K_P, 1
                    else:
                        tgt, k, sj = qM, K_Q, 0
                    # critical path: fp8 state for the next eval's L1, one hop
                    nc.vector.scalar_tensor_tensor(st_dr[sj][:, bs], pt[:], k,
                                                   tgt[:, bs], ALU.mult, ALU.add)
                    # k*psum staged to SBUF (frees the psum bank quickly);
                    # ACT is idle in this phase, DVE takes one per half
                    sc = scr_tile()
                    if b == 0 or jp == 1:
                        nc.scalar.activation(sc[:], pt[:], AF.Copy, scale=k)
                    else:
                        nc.vector.tensor_scalar_mul(sc[:], pt[:], k)
                    addbacks.append((sc, tgt, bs))
            # f32 master updates off both hot engines (SBUF-only on gpsimd);
            # p first: its result is read one eval sooner than q's
            for sc, tgt, bs in sorted(addbacks, key=lambda t: t[1] is qM):
                nc.gpsimd.tensor_tensor(tgt[:, bs], tgt[:, bs], sc[:], ALU.add)

        out_sb = [sb.tile([P, F], f32, tag=f"ob{c}", name=f"ob{c}") for c in range(BC)]

        def emit_out(src, col0):
            for b in range(NBH):
                for cp in range(BC // 2):
                    c = b * (BC // 2) + cp
                    for m in range(D // P):
                        pt = psum2()
                        scol = b * BCORE + m * BH + cp * P
                        nc.tensor.transpose(pt[:, :P], src[:, scol:scol + P],
                                            ident[:])
                        nc.scalar.copy(out_sb[c][:, col0 + m * P:col0 + (m + 1) * P],
                                       pt[:, :P])

        for step in range(STEPS):
            with nc.named_scope(f"step{step}"):
                # eval A: updates p (half-kick) and q (drift)
                grad_eval(full=True)
                if step == STEPS - 1:
                    # q is final after the drift; transpose it out and start
                    # its DMA while the last eval (p-only) runs
                    emit_out(qM, 0)
                    for c in range(BC):
                        nc.sync.dma_start(OUT[c * P:(c + 1) * P, :D],
                                          out_sb[c][:, :D])
                # eval B: second half-kick on p only
                grad_eval(full=False)

        # ---------------- output: out = concat([q, p], -1), batch-major ------
        emit_out(pM, D)
        for c in range(BC):
            nc.sync.dma_start(OUT[c * P:(c + 1) * P, D:], out_sb[c][:, D:])

    _split_multi_waits(nc)
    return nc


_CACHE = {}


def _get_nc():
    if "nc" not in _CACHE:
        _CACHE["nc"] = _build()
    return _CACHE["nc"]


def kernel(x, W1, b1, W2, b2, Wo, _trace=False):
    from concourse.bass_utils import run_bass_kernel_spmd
    nc = _get_nc()
    x = np.ascontiguousarray(np.asarray(x, dtype=np.float32))
    W1 = np.ascontiguousarray(np.asarray(W1, dtype=np.float32))
    W2 = np.ascontiguousarray(np.asarray(W2, dtype=np.float32))
    Wo = np.ascontiguousarray(np.asarray(Wo, dtype=np.float32))
    B = x.shape[0]
    xf = x.reshape(NCORES, BCORE, F * 2)
    in_maps = [
        {"x": np.ascontiguousarray(xf[c]), "w1": W1, "w2": W2, "wo": Wo}
        for c in range(NCORES)
    ]
    res = run_bass_kernel_spmd(nc, in_maps, core_ids=list(range(NCORES)),
                               trace=_trace)
    out = np.concatenate([r["out"] for r in res.results], axis=0)
    if _trace:
        kernel.last_result = res
    return out


# revision 33
# speedup vs baseline: 1.0155x; 1.0021x over previous
"""Trainium2 Bass kernel for the HNN leapfrog integrator (nn_HNN_39968965657036).

Data-parallel over batch: 8192 samples -> 8 cores x 1024. All weights and
state SBUF-resident; 16 leapfrog steps x 2 gradient evals run fully on-chip.

All four matmul layers run in fp8 DoubleRow (2x contraction per MM
instruction; the gradient depends on the state only through the relu masks,
so fp8 forward error only flips near-zero mask bits). PSUM is allocated as
[128,1024] double tiles pairing the two m-chunks of each DR weight tile, so
every elementwise psum drain is a single 1024-wide op. Activation tensors
use a batch-major DR layout [ki, b*1024 + o*512 + n] so drain destinations
are contiguous. Engine balance: Scalar does relu + sigmoid-step masks +
k*psum staging copies; Vector does the g1 mask-mult and the one-hop fp8
state updates; GpSimd applies the deferred f32 master accumulations
(SBUF-only, off both hot engines).
"""
import numpy as np
from contextlib import ExitStack

import concourse.bass as bass
import concourse.mybir as mybir
import concourse.tile as tile
from concourse.masks import make_identity

D = 256          # hnn dim; state dim = 2D = 512
F = 2 * D        # 512 features
STEPS = 16
DT = 0.1
NCORES = 8
BCORE = 1024     # batch per core
NBH = 2          # batch halves per core
BH = BCORE // NBH  # 512 = moving-operand width
P = 128
FC = F // P      # 4 feature chunks
BC = BCORE // P  # 8 batch chunks

f32 = mybir.dt.float32
fp8 = mybir.dt.float8e4

SW = 16.0            # fp8 scale on W1 / W2 (keeps entries out of subnormals)
S3 = 512.0           # fp8 scale on Wo-folded W2 (L3 stationary)
K_P = -0.5 * DT / (SW * S3)   # L4 psum -> p half-kick coefficient
K_Q = DT / (SW * S3)          # L4 psum -> q drift coefficient
SIG = 2.0 ** 20      # sigmoid(SIG*x) == exact (x>0) step after fp8 rounding


def _split_multi_waits(nc):
    """walrus codegen allows at most ONE sync wait per instruction; hoist
    extras onto preceding single-wait NoOps on the same engine queue."""
    skip = {"InstAllEngineBarrier", "InstEventSemaphore"}
    ctr = 0
    for f in nc.m.functions:
        for blk in f.blocks:
            out = []
            changed = False
            for inst in blk.instructions:
                si = inst.sync_info
                if (si is not None and si.on_wait and len(si.on_wait) > 1
                        and type(inst).__name__ not in skip):
                    waits = list(si.on_wait)
                    for w in waits[:-1]:
                        ctr += 1
                        nop = mybir.InstNoOp(name=f"I-wsplit-{ctr}", ins=[], outs=[])
                        nop.engine = inst.engine
                        nop.sync_info = mybir.SyncInfo(on_wait=[w], on_update=[])
                        out.append(nop)
                    inst.sync_info = mybir.SyncInfo(
                        on_wait=[waits[-1]], on_update=list(si.on_update or []))
                    changed = True
                out.append(inst)
            if changed:
                blk.instructions = out
    return ctr


def _build():
    nc = bass.Bass(trn_type="TRN2")
    X = nc.dram_tensor("x", [BCORE, F * 2], f32, kind="ExternalInput")   # [1024, 1024]
    W1d = nc.dram_tensor("w1", [F, F], f32, kind="ExternalInput")
    W2d = nc.dram_tensor("w2", [F, F], f32, kind="ExternalInput")
    Wod = nc.dram_tensor("wo", [1, F], f32, kind="ExternalInput")
    OUT = nc.dram_tensor("out", [BCORE, F], f32, kind="ExternalOutput")

    AF = mybir.ActivationFunctionType
    ALU = mybir.AluOpType
    DR = mybir.MatmulPerfMode.DoubleRow

    with tile.TileContext(nc) as tc, ExitStack() as ctx:
        sb = ctx.enter_context(tc.tile_pool(name="sb", bufs=1))
        ps = ctx.enter_context(tc.tile_pool(name="ps", bufs=4, space="PSUM"))

        def psum2():
            # [128, 1024] f32 double tile = 2 psum banks; 4 rotating = 8 banks
            return ps.tile([P, 2 * BH], f32, tag="mm", bufs=4, name="pmm")

        # ---------------- load ----------------
        # weights first: PE's first work (weight transposes) depends on them
        w1_sb = [sb.tile([P, F], f32, tag=f"w1_{k}", name=f"w1_{k}") for k in range(FC)]
        w2_sb = [sb.tile([P, F], f32, tag=f"w2_{k}", name=f"w2_{k}") for k in range(FC)]
        for k in range(FC):
            nc.sync.dma_start(w1_sb[k][:], W1d[k * P:(k + 1) * P, :])
            nc.sync.dma_start(w2_sb[k][:], W2d[k * P:(k + 1) * P, :])
        woT = [sb.tile([P, 1], f32, tag=f"wo{k}", name=f"wo{k}") for k in range(FC)]
        for k in range(FC):
            nc.sync.dma_start(woT[k][:], Wod[:, k * P:(k + 1) * P])
        x_sb = [sb.tile([P, F * 2], f32, tag=f"x{c}", name=f"x{c}") for c in range(BC)]
        for c in range(BC):
            nc.sync.dma_start(x_sb[c][:], X[c * P:(c + 1) * P, :])

        ident = sb.tile([P, P], f32, tag="ident")
        make_identity(nc, ident[:])

        # ---------------- weight prep: fp8 DoubleRow stationaries ----------
        # DR layout pairs feature chunks (2j, 2j+1): tile[ki, o*W + m] holds
        # element [feature f = j*256 + o*128 + ki, m].
        w1T_dr = [sb.tile([P, 2 * F], fp8, tag=f"w1T{j}", name=f"w1T{j}") for j in range(2)]
        w2T_dr = [sb.tile([P, 2 * F], fp8, tag=f"w2T{j}", name=f"w2T{j}") for j in range(2)]
        for fc in range(FC):           # source column chunk of W (feature f)
            j, o = fc // 2, fc % 2
            for mc in range(FC):       # source row chunk of W (output m)
                pt = psum2()
                nc.tensor.transpose(pt[:, :P], w1_sb[mc][:, fc * P:(fc + 1) * P],
                                    ident[:])
                nc.scalar.activation(
                    w1T_dr[j][:, o * F + mc * P:o * F + (mc + 1) * P],
                    pt[:, :P], AF.Copy, scale=SW)
                pt2 = psum2()
                nc.tensor.transpose(pt2[:, :P], w2_sb[mc][:, fc * P:(fc + 1) * P],
                                    ident[:])
                nc.scalar.activation(
                    w2T_dr[j][:, o * F + mc * P:o * F + (mc + 1) * P],
                    pt2[:, :P], AF.Copy, scale=SW)
        # L3 lhsT: w2w_dr[j][ki, o*F+i] = Wo[f]*W2[f, i]*S3 (f = row index)
        w2w_dr = [sb.tile([P, 2 * F], fp8, tag=f"w2w{j}", name=f"w2w{j}")
                  for j in range(2)]
        for c in range(FC):
            j, o = c // 2, c % 2
            nc.vector.tensor_scalar(w2w_dr[j][:, o * F:(o + 1) * F], w2_sb[c][:],
                                    woT[c][:], S3, ALU.mult, ALU.mult)
        # L4 lhsT: w1_dr[j][ki, o*F+m] = W1[f, m] * SW (f = row index)
        w1_dr = [sb.tile([P, 2 * F], fp8, tag=f"w1f{j}", name=f"w1f{j}")
                 for j in range(2)]
        for c in range(FC):
            j, o = c // 2, c % 2
            nc.vector.tensor_scalar_mul(w1_dr[j][:, o * F:(o + 1) * F],
                                        w1_sb[c][:], SW)

        # HAM warm-up: ~3.5us of dense dummy matmuls while the PE would idle
        # waiting for the x DMA; output is never read.
        warm = psum2()
        for i in range(16):
            nc.tensor.matmul(warm[:, :BH], w1T_dr[0][:, :P], w1T_dr[1][:, :BH],
                             start=(i == 0), stop=(i == 15))

        # ------- input prep: q = x[:,:,3], p = x[:,:,3]-x[:,:,2] ------------
        # masters, batch-major: [ki, b*1024 + mloc*512 + n] = state[mloc*128+ki,
        # b*512 + n]; matches st_dr's fp8 DR layout element-for-element.
        qM = sb.tile([P, 2 * BCORE], f32, tag="qM", name="qM")
        pM = sb.tile([P, 2 * BCORE], f32, tag="pM", name="pM")
        for c in range(BC):
            b, cp = c // (BC // 2), c % (BC // 2)
            xv = x_sb[c][:].rearrange("p (f c) -> p f c", c=4)
            qb = sb.tile([P, D], f32, tag="qb", bufs=3)
            pb = sb.tile([P, D], f32, tag="pb", bufs=3)
            nc.vector.tensor_copy(qb[:], xv[:, :, 3])
            nc.vector.tensor_tensor(pb[:], xv[:, :, 3], xv[:, :, 2],
                                    ALU.subtract)
            for m in range(D // P):
                col = b * BCORE + m * BH + cp * P
                pt = psum2()
                nc.tensor.transpose(pt[:, :P], qb[:, m * P:(m + 1) * P], ident[:])
                nc.scalar.copy(qM[:, col:col + P], pt[:, :P])
                pt2 = psum2()
                nc.tensor.transpose(pt2[:, :P], pb[:, m * P:(m + 1) * P], ident[:])
                nc.scalar.copy(pM[:, col:col + P], pt2[:, :P])
            # keep-alive blip: transposes don't count as PE activity for the
            # HAM clock gate, so pulse a tiny matmul as each x chunk lands
            blip = psum2()
            nc.tensor.matmul(blip[:, :64], w1T_dr[0][:, :P],
                             w1T_dr[1][:, :64], start=True, stop=True)

        # fp8 state in DR layout: st_dr[0] = q chunks, st_dr[1] = p chunks
        # (filled per batch-half so step0's L1(b0) starts on half the input)
        st_dr = [sb.tile([P, 2 * BCORE], fp8, tag=f"st{j}", name=f"st{j}")
                 for j in range(2)]
        for b in range(NBH):
            bs = slice(b * BCORE, (b + 1) * BCORE)
            nc.scalar.copy(st_dr[0][:, bs], qM[:, bs])
            nc.scalar.copy(st_dr[1][:, bs], pM[:, bs])

        a1_dr = [sb.tile([P, 2 * BCORE], fp8, tag=f"a1_{j}", name=f"a1_{j}")
                 for j in range(2)]
        m2_dr = [sb.tile([P, 2 * BCORE], fp8, tag=f"m2_{j}", name=f"m2_{j}")
                 for j in range(2)]
        g1_dr = [sb.tile([P, 2 * BCORE], fp8, tag=f"g1_{j}", name=f"g1_{j}")
                 for j in range(2)]

        def scr_tile():
            # f32 staging for k*psum so the master add runs off-psum on gpsimd
            return sb.tile([P, 2 * BH], f32, tag="scr", bufs=4, name="scr")

        # ---------------- 16 leapfrog steps ----------------
        def mm_pair(lhsT_dr, rhs_dr, jp, b):
            """One [128,1024] double psum: output chunks m=2jp, 2jp+1 for
            batch half b, each over the full 512 contraction (2 DR MMs)."""
            pt = psum2()
            bs = slice(b * BCORE, (b + 1) * BCORE)
            for o in range(2):
                m = 2 * jp + o
                half = pt[:, o * BH:(o + 1) * BH]
                for jc in range(2):
                    lhsT = lhsT_dr[jc][:].rearrange("p (o m) -> p o m", o=2)[
                        :, :, m * P:(m + 1) * P]
                    rhs = rhs_dr[jc][:, bs].rearrange("p (o n) -> p o n", o=2)
                    nc.tensor.matmul(half, lhsT, rhs, start=(jc == 0),
                                     stop=(jc == 1), perf_mode=DR)
            return pt

        def grad_eval(full, last=False):
            """One gradient eval; full=True also produces q updates.
            last=True: final eval -- update masters directly (shortest path
            to the output emit) and skip the fp8-state/scratch machinery."""
            for b in range(NBH):
                bs = slice(b * BCORE, (b + 1) * BCORE)
                for jp in range(2):  # L1: h1.T = W1 @ state.T (psum = SW*h1)
                    pt = mm_pair(w1T_dr, st_dr, jp, b)
                    nc.scalar.activation(a1_dr[jp][:, bs], pt[:], AF.Relu,
                                         scale=1.0 / SW)
            for b in range(NBH):
                bs = slice(b * BCORE, (b + 1) * BCORE)
                for jp in range(2):  # L2: h2.T = W2 @ a1.T (sign only)
                    pt = mm_pair(w2T_dr, a1_dr, jp, b)
                    if b == 1 and jp == 1:  # one per eval on DVE for balance
                        nc.vector.tensor_scalar(m2_dr[jp][:, bs], pt[:], 0.0,
                                                None, ALU.is_gt)
                    else:
                        # sigmoid(2^20 x) == exact 0/1 step after fp8 rounding
                        nc.scalar.activation(m2_dr[jp][:, bs], pt[:], AF.Sigmoid,
                                             scale=SIG)
            for b in range(NBH):
                bs = slice(b * BCORE, (b + 1) * BCORE)
                for jp in range(2):  # L3: u.T = (Wo*W2).T @ m2.T (psum = S3*u)
                    pt = mm_pair(w2w_dr, m2_dr, jp, b)
                    # g1 = (a1 > 0) * u
                    nc.vector.scalar_tensor_tensor(g1_dr[jp][:, bs],
                                                   a1_dr[jp][:, bs], 0.0, pt[:],
                                                   ALU.is_gt, ALU.mult)
            addbacks = []
            for b in range(NBH):
                bs = slice(b * BCORE, (b + 1) * BCORE)
                for jp in range(2) if full else range(1):
                    # L4: dH.T = W1.T @ g1.T (psum = SW*S3*dH)
                    # jp 0: dH_q -> p half-kick ; jp 1: dH_p -> q drift
                    pt = mm_pair(w1_dr, g1_dr, jp, b)
                    if jp == 0:
                        tgt, k, sj = pM, K_P, 1
                    else:
                        tgt, k, sj = qM, K_Q, 0
                    if last:
                        nc.vector.scalar_tensor_tensor(tgt[:, bs], pt[:], k,
                                                       tgt[:, bs], ALU.mult,
                                                       ALU.add)
                        continue
                    # critical path: fp8 state for the next eval's L1, one hop
                    nc.vector.scalar_tensor_tensor(st_dr[sj][:, bs], pt[:], k,
                                                   tgt[:, bs], ALU.mult, ALU.add)
                    # k*psum staged to SBUF (frees the psum bank quickly);
                    # ACT is idle in this phase, DVE takes one per half
                    sc = scr_tile()
                    if b == 0 or jp == 1:
                        nc.scalar.activation(sc[:], pt[:], AF.Copy, scale=k)
                    else:
                        nc.vector.tensor_scalar_mul(sc[:], pt[:], k)
                    addbacks.append((sc, tgt, bs))
            # f32 master updates off both hot engines (SBUF-only on gpsimd);
            # p first: its result is read one eval sooner than q's
            for sc, tgt, bs in sorted(addbacks, key=lambda t: t[1] is qM):
                nc.gpsimd.tensor_tensor(tgt[:, bs], tgt[:, bs], sc[:], ALU.add)

        out_sb = [sb.tile([P, F], f32, tag=f"ob{c}", name=f"ob{c}") for c in range(BC)]

        def emit_out(src, col0):
            for b in range(NBH):
                for cp in range(BC // 2):
                    c = b * (BC // 2) + cp
                    for m in range(D // P):
                        pt = psum2()
                        scol = b * BCORE + m * BH + cp * P
                        nc.tensor.transpose(pt[:, :P], src[:, scol:scol + P],
                                            ident[:])
                        nc.scalar.copy(out_sb[c][:, col0 + m * P:col0 + (m + 1) * P],
                                       pt[:, :P])

        for step in range(STEPS):
            with nc.named_scope(f"step{step}"):
                # eval A: updates p (half-kick) and q (drift)
                grad_eval(full=True)
                if step == STEPS - 1:
                    # q is final after the drift; transpose it out and start
                    # its DMA while the last eval (p-only) runs
                    emit_out(qM, 0)
                    for c in range(BC):
                        nc.sync.dma_start(OUT[c * P:(c + 1) * P, :D],
                                          out_sb[c][:, :D])
                # eval B: second half-kick on p only
                grad_eval(full=False, last=(step == STEPS - 1))

        # ---------------- output: out = concat([q, p], -1), batch-major ------
        emit_out(pM, D)
        for c in range(BC):
            nc.sync.dma_start(OUT[c * P:(c + 1) * P, D:], out_sb[c][:, D:])

    _split_multi_waits(nc)
    return nc


_CACHE = {}


def _get_nc():
    if "nc" not in _CACHE:
        _CACHE["nc"] = _build()
    return _CACHE["nc"]


def kernel(x, W1, b1, W2, b2, Wo, _trace=False):
    from concourse.bass_utils import run_bass_kernel_spmd
    nc = _get_nc()
    x = np.ascontiguousarray(np.asarray(x, dtype=np.float32))
    W1 = np.ascontiguousarray(np.asarray(W1, dtype=np.float32))
    W2 = np.ascontiguousarray(np.asarray(W2, dtype=np.float32))
    Wo = np.ascontiguousarray(np.asarray(Wo, dtype=np.float32))
    B = x.shape[0]
    xf = x.reshape(NCORES, BCORE, F * 2)
    in_maps = [
        {"x": np.ascontiguousarray(xf[c]), "w1": W1, "w2": W2, "wo": Wo}
        for c in range(NCORES)
    ]
    res = run_bass_kernel_spmd(nc, in_maps, core_ids=list(range(NCORES)),
                               trace=_trace)
    out = np.concatenate([r["out"] for r in res.results], axis=0)
    if _trace:
        kernel.last_result = res
    return out


# revision 35
# speedup vs baseline: 1.0172x; 1.0016x over previous
"""Trainium2 Bass kernel for the HNN leapfrog integrator (nn_HNN_39968965657036).

Data-parallel over batch: 8192 samples -> 8 cores x 1024. All weights and
state SBUF-resident; 16 leapfrog steps x 2 gradient evals run fully on-chip.

All four matmul layers run in fp8 DoubleRow (2x contraction per MM
instruction; the gradient depends on the state only through the relu masks,
so fp8 forward error only flips near-zero mask bits). PSUM is allocated as
[128,1024] double tiles pairing the two m-chunks of each DR weight tile, so
every elementwise psum drain is a single 1024-wide op. Activation tensors
use a batch-major DR layout [ki, b*1024 + o*512 + n] so drain destinations
are contiguous. Engine balance: Scalar does relu + sigmoid-step masks +
k*psum staging copies; Vector does the g1 mask-mult and the one-hop fp8
state updates; GpSimd applies the deferred f32 master accumulations
(SBUF-only, off both hot engines).
"""
import numpy as np
from contextlib import ExitStack

import concourse.bass as bass
import concourse.mybir as mybir
import concourse.tile as tile
from concourse.masks import make_identity

D = 256          # hnn dim; state dim = 2D = 512
F = 2 * D        # 512 features
STEPS = 16
DT = 0.1
NCORES = 8
BCORE = 1024     # batch per core
NBH = 2          # batch halves per core
BH = BCORE // NBH  # 512 = moving-operand width
P = 128
FC = F // P      # 4 feature chunks
BC = BCORE // P  # 8 batch chunks

f32 = mybir.dt.float32
fp8 = mybir.dt.float8e4

SW = 16.0            # fp8 scale on W1 / W2 (keeps entries out of subnormals)
S3 = 512.0           # fp8 scale on Wo-folded W2 (L3 stationary)
K_P = -0.5 * DT / (SW * S3)   # L4 psum -> p half-kick coefficient
K_Q = DT / (SW * S3)          # L4 psum -> q drift coefficient
SIG = 2.0 ** 20      # sigmoid(SIG*x) == exact (x>0) step after fp8 rounding


def _split_multi_waits(nc):
    """walrus codegen allows at most ONE sync wait per instruction; hoist
    extras onto preceding single-wait NoOps on the same engine queue."""
    skip = {"InstAllEngineBarrier", "InstEventSemaphore"}
    ctr = 0
    for f in nc.m.functions:
        for blk in f.blocks:
            out = []
            changed = False
            for inst in blk.instructions:
                si = inst.sync_info
                if (si is not None and si.on_wait and len(si.on_wait) > 1
                        and type(inst).__name__ not in skip):
                    waits = list(si.on_wait)
                    for w in waits[:-1]:
                        ctr += 1
                        nop = mybir.InstNoOp(name=f"I-wsplit-{ctr}", ins=[], outs=[])
                        nop.engine = inst.engine
                        nop.sync_info = mybir.SyncInfo(on_wait=[w], on_update=[])
                        out.append(nop)
                    inst.sync_info = mybir.SyncInfo(
                        on_wait=[waits[-1]], on_update=list(si.on_update or []))
                    changed = True
                out.append(inst)
            if changed:
                blk.instructions = out
    return ctr


def _build():
    nc = bass.Bass(trn_type="TRN2")
    X = nc.dram_tensor("x", [BCORE, F * 2], f32, kind="ExternalInput")   # [1024, 1024]
    W1d = nc.dram_tensor("w1", [F, F], f32, kind="ExternalInput")
    W2d = nc.dram_tensor("w2", [F, F], f32, kind="ExternalInput")
    Wod = nc.dram_tensor("wo", [1, F], f32, kind="ExternalInput")
    OUT = nc.dram_tensor("out", [BCORE, F], f32, kind="ExternalOutput")

    AF = mybir.ActivationFunctionType
    ALU = mybir.AluOpType
    DR = mybir.MatmulPerfMode.DoubleRow

    with tile.TileContext(nc) as tc, ExitStack() as ctx:
        sb = ctx.enter_context(tc.tile_pool(name="sb", bufs=1))
        ps = ctx.enter_context(tc.tile_pool(name="ps", bufs=4, space="PSUM"))

        def psum2():
            # [128, 1024] f32 double tile = 2 psum banks; 4 rotating = 8 banks
            return ps.tile([P, 2 * BH], f32, tag="mm", bufs=4, name="pmm")

        # ---------------- load ----------------
        # weights first: PE's first work (weight transposes) depends on them
        w1_sb = [sb.tile([P, F], f32, tag=f"w1_{k}", name=f"w1_{k}") for k in range(FC)]
        w2_sb = [sb.tile([P, F], f32, tag=f"w2_{k}", name=f"w2_{k}") for k in range(FC)]
        for k in range(FC):
            nc.sync.dma_start(w1_sb[k][:], W1d[k * P:(k + 1) * P, :])
            nc.sync.dma_start(w2_sb[k][:], W2d[k * P:(k + 1) * P, :])
        woT = [sb.tile([P, 1], f32, tag=f"wo{k}", name=f"wo{k}") for k in range(FC)]
        for k in range(FC):
            nc.sync.dma_start(woT[k][:], Wod[:, k * P:(k + 1) * P])
        x_sb = [sb.tile([P, F * 2], f32, tag=f"x{c}", name=f"x{c}") for c in range(BC)]
        for c in range(BC):
            nc.sync.dma_start(x_sb[c][:], X[c * P:(c + 1) * P, :])

        ident = sb.tile([P, P], f32, tag="ident")
        make_identity(nc, ident[:])

        # ---------------- weight prep: fp8 DoubleRow stationaries ----------
        # DR layout pairs feature chunks (2j, 2j+1): tile[ki, o*W + m] holds
        # element [feature f = j*256 + o*128 + ki, m].
        w1T_dr = [sb.tile([P, 2 * F], fp8, tag=f"w1T{j}", name=f"w1T{j}") for j in range(2)]
        w2T_dr = [sb.tile([P, 2 * F], fp8, tag=f"w2T{j}", name=f"w2T{j}") for j in range(2)]
        for fc in range(FC):           # source column chunk of W (feature f)
            j, o = fc // 2, fc % 2
            for mc in range(FC):       # source row chunk of W (output m)
                pt = psum2()
                nc.tensor.transpose(pt[:, :P], w1_sb[mc][:, fc * P:(fc + 1) * P],
                                    ident[:])
                nc.scalar.activation(
                    w1T_dr[j][:, o * F + mc * P:o * F + (mc + 1) * P],
                    pt[:, :P], AF.Copy, scale=SW)
                pt2 = psum2()
                nc.tensor.transpose(pt2[:, :P], w2_sb[mc][:, fc * P:(fc + 1) * P],
                                    ident[:])
                nc.scalar.activation(
                    w2T_dr[j][:, o * F + mc * P:o * F + (mc + 1) * P],
                    pt2[:, :P], AF.Copy, scale=SW)
        # L3 lhsT: w2w_dr[j][ki, o*F+i] = Wo[f]*W2[f, i]*S3 (f = row index)
        w2w_dr = [sb.tile([P, 2 * F], fp8, tag=f"w2w{j}", name=f"w2w{j}")
                  for j in range(2)]
        for c in range(FC):
            j, o = c // 2, c % 2
            nc.vector.tensor_scalar(w2w_dr[j][:, o * F:(o + 1) * F], w2_sb[c][:],
                                    woT[c][:], S3, ALU.mult, ALU.mult)
        # L4 lhsT: w1_dr[j][ki, o*F+m] = W1[f, m] * SW (f = row index)
        w1_dr = [sb.tile([P, 2 * F], fp8, tag=f"w1f{j}", name=f"w1f{j}")
                 for j in range(2)]
        for c in range(FC):
            j, o = c // 2, c % 2
            nc.vector.tensor_scalar_mul(w1_dr[j][:, o * F:(o + 1) * F],
                                        w1_sb[c][:], SW)



        # ------- input prep: q = x[:,:,3], p = x[:,:,3]-x[:,:,2] ------------
        # masters, batch-major: [ki, b*1024 + mloc*512 + n] = state[mloc*128+ki,
        # b*512 + n]; matches st_dr's fp8 DR layout element-for-element.
        qM = sb.tile([P, 2 * BCORE], f32, tag="qM", name="qM")
        pM = sb.tile([P, 2 * BCORE], f32, tag="pM", name="pM")
        for c in range(BC):
            b, cp = c // (BC // 2), c % (BC // 2)
            xv = x_sb[c][:].rearrange("p (f c) -> p f c", c=4)
            qb = sb.tile([P, D], f32, tag="qb", bufs=3)
            pb = sb.tile([P, D], f32, tag="pb", bufs=3)
            nc.vector.tensor_copy(qb[:], xv[:, :, 3])
            nc.vector.tensor_tensor(pb[:], xv[:, :, 3], xv[:, :, 2],
                                    ALU.subtract)
            for m in range(D // P):
                col = b * BCORE + m * BH + cp * P
                pt = psum2()
                nc.tensor.transpose(pt[:, :P], qb[:, m * P:(m + 1) * P], ident[:])
                nc.scalar.copy(qM[:, col:col + P], pt[:, :P])
                pt2 = psum2()
                nc.tensor.transpose(pt2[:, :P], pb[:, m * P:(m + 1) * P], ident[:])
                nc.scalar.copy(pM[:, col:col + P], pt2[:, :P])
            # keep-alive blip: transposes don't count as PE activity for the
            # HAM clock gate, so pulse a tiny matmul as each x chunk lands
            blip = psum2()
            nc.tensor.matmul(blip[:, :64], w1T_dr[0][:, :P],
                             w1T_dr[1][:, :64], start=True, stop=True)
            if c == BC // 2 - 1:
                # HAM warm-up: ~3.5us of dense dummy matmuls in the gap where
                # the PE would idle waiting for x's second half; never read.
                warm = psum2()
                for i in range(16):
                    nc.tensor.matmul(warm[:, :BH], w1T_dr[0][:, :P],
                                     w1T_dr[1][:, :BH], start=(i == 0),
                                     stop=(i == 15))

        # fp8 state in DR layout: st_dr[0] = q chunks, st_dr[1] = p chunks
        # (filled per batch-half so step0's L1(b0) starts on half the input)
        st_dr = [sb.tile([P, 2 * BCORE], fp8, tag=f"st{j}", name=f"st{j}")
                 for j in range(2)]
        for b in range(NBH):
            bs = slice(b * BCORE, (b + 1) * BCORE)
            nc.scalar.copy(st_dr[0][:, bs], qM[:, bs])
            nc.scalar.copy(st_dr[1][:, bs], pM[:, bs])

        a1_dr = [sb.tile([P, 2 * BCORE], fp8, tag=f"a1_{j}", name=f"a1_{j}")
                 for j in range(2)]
        m2_dr = [sb.tile([P, 2 * BCORE], fp8, tag=f"m2_{j}", name=f"m2_{j}")
                 for j in range(2)]
        g1_dr = [sb.tile([P, 2 * BCORE], fp8, tag=f"g1_{j}", name=f"g1_{j}")
                 for j in range(2)]

        def scr_tile():
            # f32 staging for k*psum so the master add runs off-psum on gpsimd
            return sb.tile([P, 2 * BH], f32, tag="scr", bufs=4, name="scr")

        # ---------------- 16 leapfrog steps ----------------
        def mm_pair(lhsT_dr, rhs_dr, jp, b):
            """One [128,1024] double psum: output chunks m=2jp, 2jp+1 for
            batch half b, each over the full 512 contraction (2 DR MMs)."""
            pt = psum2()
            bs = slice(b * BCORE, (b + 1) * BCORE)
            for o in range(2):
                m = 2 * jp + o
                half = pt[:, o * BH:(o + 1) * BH]
                for jc in range(2):
                    lhsT = lhsT_dr[jc][:].rearrange("p (o m) -> p o m", o=2)[
                        :, :, m * P:(m + 1) * P]
                    rhs = rhs_dr[jc][:, bs].rearrange("p (o n) -> p o n", o=2)
                    nc.tensor.matmul(half, lhsT, rhs, start=(jc == 0),
                                     stop=(jc == 1), perf_mode=DR)
            return pt

        def grad_eval(full, last=False):
            """One gradient eval; full=True also produces q updates.
            last=True: final eval -- update masters directly (shortest path
            to the output emit) and skip the fp8-state/scratch machinery."""
            for b in range(NBH):
                bs = slice(b * BCORE, (b + 1) * BCORE)
                for jp in range(2):  # L1: h1.T = W1 @ state.T (psum = SW*h1)
                    pt = mm_pair(w1T_dr, st_dr, jp, b)
                    nc.scalar.activation(a1_dr[jp][:, bs], pt[:], AF.Relu,
                                         scale=1.0 / SW)
            for b in range(NBH):
                bs = slice(b * BCORE, (b + 1) * BCORE)
                for jp in range(2):  # L2: h2.T = W2 @ a1.T (sign only)
                    pt = mm_pair(w2T_dr, a1_dr, jp, b)
                    if b == 1 and jp == 1:  # one per eval on DVE for balance
                        nc.vector.tensor_scalar(m2_dr[jp][:, bs], pt[:], 0.0,
                                                None, ALU.is_gt)
                    else:
                        # sigmoid(2^20 x) == exact 0/1 step after fp8 rounding
                        nc.scalar.activation(m2_dr[jp][:, bs], pt[:], AF.Sigmoid,
                                             scale=SIG)
            for b in range(NBH):
                bs = slice(b * BCORE, (b + 1) * BCORE)
                for jp in range(2):  # L3: u.T = (Wo*W2).T @ m2.T (psum = S3*u)
                    pt = mm_pair(w2w_dr, m2_dr, jp, b)
                    # g1 = (a1 > 0) * u
                    nc.vector.scalar_tensor_tensor(g1_dr[jp][:, bs],
                                                   a1_dr[jp][:, bs], 0.0, pt[:],
                                                   ALU.is_gt, ALU.mult)
            addbacks = []
            for b in range(NBH):
                bs = slice(b * BCORE, (b + 1) * BCORE)
                for jp in range(2) if full else range(1):
                    # L4: dH.T = W1.T @ g1.T (psum = SW*S3*dH)
                    # jp 0: dH_q -> p half-kick ; jp 1: dH_p -> q drift
                    pt = mm_pair(w1_dr, g1_dr, jp, b)
                    if jp == 0:
                        tgt, k, sj = pM, K_P, 1
                    else:
                        tgt, k, sj = qM, K_Q, 0
                    if last:
                        nc.vector.scalar_tensor_tensor(tgt[:, bs], pt[:], k,
                                                       tgt[:, bs], ALU.mult,
                                                       ALU.add)
                        continue
                    # critical path: fp8 state for the next eval's L1, one hop
                    nc.vector.scalar_tensor_tensor(st_dr[sj][:, bs], pt[:], k,
                                                   tgt[:, bs], ALU.mult, ALU.add)
                    # k*psum staged to SBUF (frees the psum bank quickly);
                    # ACT is idle in this phase, DVE takes one per half
                    sc = scr_tile()
                    if b == 0 or jp == 1:
                        nc.scalar.activation(sc[:], pt[:], AF.Copy, scale=k)
                    else:
                        nc.vector.tensor_scalar_mul(sc[:], pt[:], k)
                    addbacks.append((sc, tgt, bs))
            # f32 master updates off both hot engines (SBUF-only on gpsimd);
            # p first: its result is read one eval sooner than q's
            for sc, tgt, bs in sorted(addbacks, key=lambda t: t[1] is qM):
                nc.gpsimd.tensor_tensor(tgt[:, bs], tgt[:, bs], sc[:], ALU.add)

        out_sb = [sb.tile([P, F], f32, tag=f"ob{c}", name=f"ob{c}") for c in range(BC)]

        def emit_out(src, col0):
            for b in range(NBH):
                for cp in range(BC // 2):
                    c = b * (BC // 2) + cp
                    for m in range(D // P):
                        pt = psum2()
                        scol = b * BCORE + m * BH + cp * P
                        nc.tensor.transpose(pt[:, :P], src[:, scol:scol + P],
                                            ident[:])
                        nc.scalar.copy(out_sb[c][:, col0 + m * P:col0 + (m + 1) * P],
                                       pt[:, :P])

        for step in range(STEPS):
            with nc.named_scope(f"step{step}"):
                # eval A: updates p (half-kick) and q (drift)
                grad_eval(full=True)
                if step == STEPS - 1:
                    # q is final after the drift; transpose it out and start
                    # its DMA while the last eval (p-only) runs
                    emit_out(qM, 0)
                    for c in range(BC):
                        nc.sync.dma_start(OUT[c * P:(c + 1) * P, :D],
                                          out_sb[c][:, :D])
                # eval B: second half-kick on p only
                grad_eval(full=False, last=(step == STEPS - 1))

        # ---------------- output: out = concat([q, p], -1), batch-major ------
        emit_out(pM, D)
        for c in range(BC):
            nc.sync.dma_start(OUT[c * P:(c + 1) * P, D:], out_sb[c][:, D:])

    _split_multi_waits(nc)
    return nc


_CACHE = {}


def _get_nc():
    if "nc" not in _CACHE:
        _CACHE["nc"] = _build()
    return _CACHE["nc"]


def kernel(x, W1, b1, W2, b2, Wo, _trace=False):
    from concourse.bass_utils import run_bass_kernel_spmd
    nc = _get_nc()
    x = np.ascontiguousarray(np.asarray(x, dtype=np.float32))
    W1 = np.ascontiguousarray(np.asarray(W1, dtype=np.float32))
    W2 = np.ascontiguousarray(np.asarray(W2, dtype=np.float32))
    Wo = np.ascontiguousarray(np.asarray(Wo, dtype=np.float32))
    B = x.shape[0]
    xf = x.reshape(NCORES, BCORE, F * 2)
    in_maps = [
        {"x": np.ascontiguousarray(xf[c]), "w1": W1, "w2": W2, "wo": Wo}
        for c in range(NCORES)
    ]
    res = run_bass_kernel_spmd(nc, in_maps, core_ids=list(range(NCORES)),
                               trace=_trace)
    out = np.concatenate([r["out"] for r in res.results], axis=0)
    if _trace:
        kernel.last_result = res
    return out
